# revision 1
# baseline (speedup 1.0000x reference)
"""Trainium2 Bass kernel for LocalGlobalSelfAttention.

Sharding: 8 cores = 4 batches x 2 sequence-halves (no collectives).
Each core computes, for its (batch b, half h):
  - global attention: queries = its half (SH rows), keys/values = full seq
  - local windowed attention: fully contained in its half
  - output projections (g+l accumulated in PSUM) + bias + residual + layernorm

Host side passes x^T with the core's own half FIRST (column-permuted; attention
is permutation-invariant over key positions) so the device can statically slice
queries as columns [0:SH]. Weights are host-converted to bf16. Matmuls run bf16
with fp32 PSUM accumulation. Softmax skips the max-subtraction (scores are
O(1) here) and exp() runs on ScalarE directly from PSUM; rowsums ride along the
AV matmul via a ones-column appended to V; normalization happens after AV.
"""

import numpy as np
import ml_dtypes
from collections import deque
from contextlib import ExitStack

BF16 = ml_dtypes.bfloat16

FULL_CFG = dict(S=2048, D=1024, H=16, K=64, NW=8)
N_CORES = 8
LN_EPS = 1e-3


def _chunks(total, size):
    return [(o, min(size, total - o)) for o in range(0, total, size)]


def build_nc(cfg=None):
    """Build + compile the per-core Bass program (SPMD, same on all cores)."""
    import concourse.bass as bass
    import concourse.tile as tile
    import concourse.mybir as mybir
    from concourse import bacc

    cfg = dict(cfg or FULL_CFG)
    S, D, H, K, NW = cfg["S"], cfg["D"], cfg["H"], cfg["K"], cfg["NW"]
    HK = H * K
    SH = S // 2          # per-core query rows (half the sequence)
    WIN = S // NW        # local attention window
    NWH = SH // WIN      # windows in this core's half
    assert K == 64 and D % 128 == 0 and HK % 128 == 0
    assert SH % 128 == 0 and WIN % 128 == 0 and NWH * WIN == SH

    ND = D // 128        # d-tiles
    NHK = HK // 128      # head-pair tiles (2 heads each)
    NST = S // 128       # s-tiles (full seq)
    NQT = SH // 128      # q-tiles (half seq)
    NSS = WIN // 128     # s-subtiles per window

    f32 = mybir.dt.float32
    bf16 = mybir.dt.bfloat16
    Exp = mybir.ActivationFunctionType.Exp
    Square = mybir.ActivationFunctionType.Square
    Sqrt = mybir.ActivationFunctionType.Sqrt
    add_op = mybir.AluOpType.add
    mult_op = mybir.AluOpType.mult
    sub_op = mybir.AluOpType.subtract
    AxX = mybir.AxisListType.X

    nc = bacc.Bacc("TRN2", target_bir_lowering=False, debug=False,
                   num_devices=N_CORES)

    # ---- DRAM parameters -------------------------------------------------
    xT_d = nc.dram_tensor("xT", [D, S], bf16, kind="ExternalInput")
    xq_d = nc.dram_tensor("xq", [SH, D], f32, kind="ExternalInput")
    w_d = {}
    for nm in ("wq_g", "wk_g", "wv_g", "wq_l", "wk_l", "wv_l"):
        w_d[nm] = nc.dram_tensor(nm, [D, HK], bf16, kind="ExternalInput")
    wo_g_d = nc.dram_tensor("wo_g", [HK, D], bf16, kind="ExternalInput")
    wo_l_d = nc.dram_tensor("wo_l", [HK, D], bf16, kind="ExternalInput")
    bcol_d = {}
    for nm in ("bq_g", "bk_g", "bq_l", "bk_l"):
        bcol_d[nm] = nc.dram_tensor(nm, [NHK, 128], f32, kind="ExternalInput")
    bv_g_d = nc.dram_tensor("bv_g", [1, HK], bf16, kind="ExternalInput")
    bv_l_d = nc.dram_tensor("bv_l", [1, HK], bf16, kind="ExternalInput")
    bo_d = nc.dram_tensor("bo", [1, D], bf16, kind="ExternalInput")
    gamma_d = nc.dram_tensor("gamma", [1, D], f32, kind="ExternalInput")
    beta_d = nc.dram_tensor("beta", [1, D], f32, kind="ExternalInput")
    out_d = nc.dram_tensor("out", [SH, D], f32, kind="ExternalOutput")

    # DRAM scratch for normalized o^T (bf16), per set; frees SBUF across phases
    oscr = {"g": nc.dram_tensor("oscr_g", [HK, SH], bf16),
            "l": nc.dram_tensor("oscr_l", [HK, SH], bf16)}

    PS = bass.MemorySpace.PSUM

    with tile.TileContext(nc) as tc, ExitStack() as ctx:
        # ---- small constants (live whole kernel) -------------------------
        cpool = ctx.enter_context(tc.tile_pool(name="consts", bufs=1))
        ones_bf = cpool.tile([1, 128], bf16, tag="ones", name="ones")
        nc.vector.memset(ones_bf[:], 1.0)
        eps_col = cpool.tile([128, 1], f32, tag="eps", name="eps")
        nc.vector.memset(eps_col[:], float(LN_EPS))
        brow_sb = {}
        for nm, d in (("bv_g", bv_g_d), ("bv_l", bv_l_d), ("bo", bo_d)):
            t = cpool.tile([1, d.shape[1]], bf16, tag=nm)
            nc.sync.dma_start(t[:], d[:])
            brow_sb[nm] = t
        bcol_sb = {}
        for nm, d in bcol_d.items():
            cols = []
            for j in range(NHK):
                t = cpool.tile([128, 1], f32, tag=f"{nm}{j}", name=f"{nm}{j}")
                nc.sync.dma_start(t[:], d[j, :].rearrange("(a b) -> a b", b=1))
                cols.append(t)
            bcol_sb[nm] = cols

        # ---- attention (shared for global/local) -------------------------
        def attention(kT, qT, vx, rounds, o_dst, prefix):
            """rounds: list of (segments, start, stop); segment =
            (q_off, q_len, s_col, vx_idx): scores^T for s-tile at kT column
            s_col vs queries [q_off:q_off+q_len], AV into o_ps[:, q_off:...]
            accumulated with start/stop. o_dst: DRAM [HK, SH] scratch."""
            with tc.tile_pool(name=f"{prefix}sc", bufs=2, space=PS) as scp, \
                 tc.tile_pool(name=f"{prefix}op", bufs=1, space=PS) as opp, \
                 tc.tile_pool(name=f"{prefix}ex", bufs=3) as exp_p, \
                 tc.tile_pool(name=f"{prefix}no", bufs=2) as nop:
                # start/stop must be bank-granular: `start` zeroes the whole
                # 2KB PSUM bank, so only the FIRST matmul touching a bank may
                # set it and only the LAST may stop it.
                first_b, last_b = {}, {}
                for ri, (segs, _, _) in enumerate(rounds):
                    for (qo, ql, _sc, _vx) in segs:
                        for co, cl in _chunks(ql, 512):
                            bank = (qo + co) // 512
                            first_b.setdefault(bank, (ri, qo + co))
                            last_b[bank] = (ri, qo + co)

                for hp in range(NHK):
                    o_ps = [opp.tile([65, SH], f32, tag=f"ops{sub}", name=f"ops{sub}")
                            for sub in range(2)]
                    pend = deque()

                    def do_av(item):
                        sub, ex, segs, ri = item
                        h = 2 * hp + sub
                        for (qo, ql, _scol, vxt) in segs:
                            for co, cl in _chunks(ql, 512):
                                col = qo + co
                                bank = col // 512
                                nc.tensor.matmul(
                                    o_ps[sub][:, col:col + cl],
                                    vx[vxt][:, h, :], ex[:, col:col + cl],
                                    start=(first_b[bank] == (ri, col)),
                                    stop=(last_b[bank] == (ri, col)))

                    for ri, (segs, st_, sp_) in enumerate(rounds):
                        for sub in range(2):
                            po = sub * 64
                            sc = scp.tile([128, SH], f32, tag="sc", name="sc")
                            for (qo, ql, scol, _vxt) in segs:
                                for co, cl in _chunks(ql, 512):
                                    nc.tensor.matmul(
                                        sc[:, qo + co:qo + co + cl],
                                        kT[hp][po:po + 64, scol:scol + 128],
                                        qT[hp][po:po + 64, qo + co:qo + co + cl],
                                        start=True, stop=True)
                            ex = exp_p.tile([128, SH], bf16, tag="ex", name="ex")
                            nc.scalar.activation(ex[:], sc[:], Exp, scale=0.125)
                            pend.append((sub, ex, segs, ri))
                            while len(pend) > 2:
                                do_av(pend.popleft())
                    while pend:
                        do_av(pend.popleft())

                    # normalize by rowsum (row 64) and spill to DRAM scratch
                    for sub in range(2):
                        rinv = nop.tile([1, SH], f32, tag=f"ri{sub}", name=f"ri{sub}")
                        nc.vector.reciprocal(rinv[:], o_ps[sub][64:65, :])
                        rb = nop.tile([64, SH], f32, tag=f"rb{sub}", name=f"rb{sub}")
                        nc.gpsimd.partition_broadcast(rb[:], rinv[0:1, :])
                        oh = nop.tile([64, SH], bf16, tag=f"oh{sub}", name=f"oh{sub}")
                        nc.vector.tensor_tensor(
                            oh[:], o_ps[sub][0:64, :], rb[:], mult_op)
                        nc.sync.dma_start(
                            o_dst[hp * 128 + sub * 64:hp * 128 + sub * 64 + 64, :],
                            oh[:])

        g_rounds = [([(0, SH, st * 128, st)], st == 0, st == NST - 1)
                    for st in range(NST)]
        l_rounds = [([(w * WIN, WIN, (w * NSS + ss) * 128, w * NSS + ss)
                      for w in range(NWH)], ss == 0, ss == NSS - 1)
                    for ss in range(NSS)]

        with tc.tile_pool(name="kqvl", bufs=1) as kqvl:
            kT_l = [kqvl.tile([128, SH], bf16, tag=f"ktl{j}", name=f"ktl{j}") for j in range(NHK)]
            qT_l = [kqvl.tile([128, SH], bf16, tag=f"qtl{j}", name=f"qtl{j}") for j in range(NHK)]
            vx_l = [kqvl.tile([128, H, 65], bf16, tag=f"vxl{t}", name=f"vxl{t}")
                    for t in range(SH // 128)]

            with tc.tile_pool(name="kqvg", bufs=1) as kqvg:
                kT_g = [kqvg.tile([128, S], bf16, tag=f"ktg{j}", name=f"ktg{j}")
                        for j in range(NHK)]
                qT_g = [kqvg.tile([128, SH], bf16, tag=f"qtg{j}", name=f"qtg{j}")
                        for j in range(NHK)]
                vx_g = [kqvg.tile([128, H, 65], bf16, tag=f"vxg{t}", name=f"vxg{t}")
                        for t in range(NST)]

                # ========== Phase A: projections =========================
                with tc.tile_pool(name="xin", bufs=1) as xin, \
                     tc.tile_pool(name="wt", bufs=2) as wt, \
                     tc.tile_pool(name="ppsum", bufs=2, space=PS) as ppsum:

                    xT_sb = [xin.tile([128, S], bf16, tag=f"xt{d}", name=f"xt{d}")
                             for d in range(ND)]
                    for d in range(ND):
                        nc.sync.dma_start(xT_sb[d][:],
                                          xT_d[d * 128:(d + 1) * 128, :])

                    def load_w(nm):
                        ts = []
                        for d in range(ND):
                            t = wt.tile([128, HK], bf16, tag=f"wd{d}", name=f"wd{d}")
                            nc.sync.dma_start(
                                t[:], w_d[nm][d * 128:(d + 1) * 128, :])
                            ts.append(t)
                        return ts

                    def proj_kq(w_tiles, s_len, out_tiles, bias_cols):
                        # out[hk, s] = (x @ w)^T + bias ; hk chunks of 128
                        for j in range(NHK):
                            pt = ppsum.tile([128, s_len], f32, tag="pp", name="pp")
                            for d in range(ND):
                                for so, sl in _chunks(s_len, 512):
                                    nc.tensor.matmul(
                                        pt[:, so:so + sl],
                                        w_tiles[d][:, j * 128:(j + 1) * 128],
                                        xT_sb[d][:, so:so + sl],
                                        start=(d == 0), stop=(d == ND - 1))
                            nc.vector.tensor_scalar(
                                out_tiles[j][:], pt[:], bias_cols[j], None,
                                add_op)

                    def proj_v(w_tiles, s_len, out_tiles, bias_row):
                        # out[s, hk] = x @ w + bias ; s tiles of 128
                        for t in range(s_len // 128):
                            pt = ppsum.tile([128, HK], f32, tag="pp", name="pp")
                            for d in range(ND):
                                for ho, hl in _chunks(HK, 512):
                                    nc.tensor.matmul(
                                        pt[:, ho:ho + hl],
                                        xT_sb[d][:, t * 128:(t + 1) * 128],
                                        w_tiles[d][:, ho:ho + hl],
                                        start=(d == 0), stop=False)
                            for ho, hl in _chunks(HK, 512):
                                nc.tensor.matmul(
                                    pt[:, ho:ho + hl], ones_bf[0:1, 0:128],
                                    bias_row[0:1, ho:ho + hl],
                                    start=False, stop=True)
                            ot = out_tiles[t]
                            nc.vector.memset(ot[:, :, 64:65], 1.0)
                            nc.vector.tensor_copy(
                                ot[:, :, 0:64],
                                pt[:].rearrange("p (h k) -> p h k", k=64))

                    wts = load_w("wk_g")
                    proj_kq(wts, S, kT_g, bcol_sb["bk_g"])
                    wts = load_w("wq_g")
                    proj_kq(wts, SH, qT_g, bcol_sb["bq_g"])
                    wts = load_w("wv_g")
                    proj_v(wts, S, vx_g, brow_sb["bv_g"])
                    wts = load_w("wk_l")
                    proj_kq(wts, SH, kT_l, bcol_sb["bk_l"])
                    wts = load_w("wq_l")
                    proj_kq(wts, SH, qT_l, bcol_sb["bq_l"])
                    wts = load_w("wv_l")
                    proj_v(wts, SH, vx_l, brow_sb["bv_l"])

                # ========== Phase B: global attention ====================
                attention(kT_g, qT_g, vx_g, g_rounds, oscr["g"], "g")

            # ========== Phase C: local attention =========================
            attention(kT_l, qT_l, vx_l, l_rounds, oscr["l"], "l")

        # ========== Phase D: output projection + residual + layernorm ====
        with tc.tile_pool(name="wo", bufs=1) as wop, \
             tc.tile_pool(name="opd", bufs=1) as opd, \
             tc.tile_pool(name="ypsum", bufs=2, space=PS) as ypp, \
             tc.tile_pool(name="ln", bufs=2) as lnp:
            gamma_bc = lnp.tile([128, D], f32, tag="gamma", name="gamma", bufs=1)
            nc.sync.dma_start(gamma_bc[:], gamma_d[:].partition_broadcast(128))
            beta_bc = lnp.tile([128, D], f32, tag="beta", name="beta", bufs=1)
            nc.sync.dma_start(beta_bc[:], beta_d[:].partition_broadcast(128))

            wo_sb, o_sb = {}, {}
            for st_, d in (("g", wo_g_d), ("l", wo_l_d)):
                wo_sb[st_] = [wop.tile([128, D], bf16, tag=f"wo{st_}{t}", name=f"wo{st_}{t}")
                              for t in range(NHK)]
                o_sb[st_] = [opd.tile([128, SH], bf16, tag=f"ob{st_}{t}", name=f"ob{st_}{t}")
                             for t in range(NHK)]
                for t in range(NHK):
                    nc.sync.dma_start(wo_sb[st_][t][:],
                                      d[t * 128:(t + 1) * 128, :])
                    nc.sync.dma_start(o_sb[st_][t][:],
                                      oscr[st_][t * 128:(t + 1) * 128, :])
            for qt in range(NQT):
                ps_y = ypp.tile([128, D], f32, tag="py", name="py")
                for do, dl in _chunks(D, 512):
                    first = True
                    for st_ in ("g", "l"):
                        for t in range(NHK):
                            nc.tensor.matmul(
                                ps_y[:, do:do + dl],
                                o_sb[st_][t][:, qt * 128:(qt + 1) * 128],
                                wo_sb[st_][t][:, do:do + dl],
                                start=first, stop=False)
                            first = False
                    nc.tensor.matmul(
                        ps_y[:, do:do + dl], ones_bf[0:1, 0:128],
                        brow_sb["bo"][0:1, do:do + dl], start=False, stop=True)
                xq_t = lnp.tile([128, D], f32, tag="xq", name="xq")
                nc.sync.dma_start(xq_t[:], xq_d[qt * 128:(qt + 1) * 128, :])
                y = lnp.tile([128, D], f32, tag="y", name="y")
                nc.vector.tensor_tensor(y[:], ps_y[:], xq_t[:], add_op)
                ssum = lnp.tile([128, 1], f32, tag="ssum", name="ssum")
                nc.vector.reduce_sum(ssum[:], y[:], axis=AxX)
                sqd = lnp.tile([128, D], bf16, tag="sqd", name="sqd")
                ssq = lnp.tile([128, 1], f32, tag="ssq", name="ssq")
                nc.scalar.activation(sqd[:], y[:], Square, accum_out=ssq[:])
                mu = lnp.tile([128, 1], f32, tag="mu", name="mu")
                nc.vector.tensor_scalar_mul(mu[:], ssum[:], 1.0 / D)
                var = lnp.tile([128, 1], f32, tag="var", name="var")
                nc.vector.tensor_scalar_mul(var[:], ssq[:], 1.0 / D)
                mu2 = lnp.tile([128, 1], f32, tag="mu2", name="mu2")
                nc.vector.tensor_tensor(mu2[:], mu[:], mu[:], mult_op)
                nc.vector.tensor_tensor(var[:], var[:], mu2[:], sub_op)
                sd = lnp.tile([128, 1], f32, tag="sd", name="sd")
                nc.scalar.activation(sd[:], var[:], Sqrt, bias=eps_col[:])
                rstd = lnp.tile([128, 1], f32, tag="rstd", name="rstd")
                nc.vector.reciprocal(rstd[:], sd[:])
                bco = lnp.tile([128, 1], f32, tag="bco", name="bco")
                nc.vector.tensor_tensor(bco[:], mu[:], rstd[:], mult_op)
                nc.vector.tensor_scalar_mul(bco[:], bco[:], -1.0)
                t1 = lnp.tile([128, D], f32, tag="t1", name="t1")
                nc.vector.tensor_scalar(t1[:], y[:], rstd[:], bco[:],
                                        mult_op, add_op)
                t2 = lnp.tile([128, D], f32, tag="t2", name="t2")
                nc.vector.tensor_tensor(t2[:], t1[:], gamma_bc[:], mult_op)
                ot = lnp.tile([128, D], f32, tag="ot", name="ot")
                nc.vector.tensor_tensor(ot[:], t2[:], beta_bc[:], add_op)
                nc.sync.dma_start(out_d[qt * 128:(qt + 1) * 128, :], ot[:])

    nc.compile()
    return nc


def make_in_maps(inputs, cfg=None):
    """Build per-core input maps from the full (unsharded) problem inputs."""
    cfg = dict(cfg or FULL_CFG)
    S, D, H, K = cfg["S"], cfg["D"], cfg["H"], cfg["K"]
    HK = H * K
    SH = S // 2
    NHK = HK // 128

    def np32(a):
        return np.asarray(a, dtype=np.float32)

    shared = {}
    for nm, key in (("wq_g", "gWq"), ("wk_g", "gWk"), ("wv_g", "gWv"),
                    ("wq_l", "lWq"), ("wk_l", "lWk"), ("wv_l", "lWv")):
        shared[nm] = np.ascontiguousarray(
            np32(inputs[key]).reshape(D, HK)).astype(BF16)
    shared["wo_g"] = np.ascontiguousarray(
        np32(inputs["gWo"]).reshape(HK, D)).astype(BF16)
    shared["wo_l"] = np.ascontiguousarray(
        np32(inputs["lWo"]).reshape(HK, D)).astype(BF16)
    for nm, key in (("bq_g", "gbq"), ("bk_g", "gbk"),
                    ("bq_l", "lbq"), ("bk_l", "lbk")):
        shared[nm] = np.ascontiguousarray(np32(inputs[key]).reshape(NHK, 128))
    shared["bv_g"] = np32(inputs["gbv"]).reshape(1, HK).astype(BF16)
    shared["bv_l"] = np32(inputs["lbv"]).reshape(1, HK).astype(BF16)
    shared["bo"] = (np32(inputs["gbo"]) +
                    np32(inputs["lbo"])).reshape(1, D).astype(BF16)
    shared["gamma"] = np32(inputs["gamma"]).reshape(1, D)
    shared["beta"] = np32(inputs["beta"]).reshape(1, D)

    x = np32(inputs["x"])
    in_maps = []
    for c in range(N_CORES):
        b, half = divmod(c, 2)
        xb = x[b]
        # own half first (queries/local), other half second; global attention
        # is invariant to key/value column order
        xperm = np.concatenate([xb[half * SH:(half + 1) * SH],
                                xb[(1 - half) * SH:(2 - half) * SH]], axis=0)
        m = dict(shared)
        m["xT"] = np.ascontiguousarray(xperm.T).astype(BF16)
        m["xq"] = np.ascontiguousarray(xperm[0:SH])
        in_maps.append(m)
    return in_maps


def assemble_out(results, cfg=None):
    cfg = dict(cfg or FULL_CFG)
    S, D = cfg["S"], cfg["D"]
    SH = S // 2
    B = N_CORES // 2
    out = np.empty((B, S, D), np.float32)
    for c in range(N_CORES):
        b, half = divmod(c, 2)
        out[b, half * SH:(half + 1) * SH] = results[c]["out"]
    return out


_NC_CACHE = {}


def kernel(**inputs):
    from concourse.bass_utils import run_bass_kernel_spmd
    if "nc" not in _NC_CACHE:
        _NC_CACHE["nc"] = build_nc()
    nc = _NC_CACHE["nc"]
    in_maps = make_in_maps(inputs)
    res = run_bass_kernel_spmd(nc, in_maps, list(range(N_CORES)))
    return assemble_out(res.results)



# revision 13
# speedup vs baseline: 1.6082x; 1.6082x over previous
"""Trainium2 Bass kernel for LocalGlobalSelfAttention (fp8 DoubleRow).

Sharding: 8 cores = 4 batches x 2 sequence-halves (no collectives).
Each core computes, for its (batch b, half h):
  - global attention: queries = its half (SH rows), keys/values = full seq
  - local windowed attention: fully contained in its half
  - output projections (g+l accumulated in PSUM) + residual + layernorm

All matmuls run fp8e4 with fp32 PSUM accumulation; contraction-128 matmuls
(projections, AV, output projection) use DoubleRow (2x throughput). Scores
(K=64 contraction) run plain fp8. Scaling ledger: weights are host-scaled
x32 (fp8 subnormal avoidance), un-scaled by 1/32 in the PSUM->SBUF cast;
attention outputs carry x64 (ones-column = 1/64 so rowsum reciprocal yields
64/sum); the final combine divides by 64*32 = 2048. Softmax skips max
subtraction (scores are O(1)); rowsums ride the AV matmul via the ones
column; reciprocal uses the fast approx DVE op.
"""

import numpy as np
import ml_dtypes
from collections import deque
from contextlib import ExitStack

F8 = ml_dtypes.float8_e4m3

FULL_CFG = dict(S=2048, D=1024, H=16, K=64, NW=8)
N_CORES = 8
LN_EPS = 1e-3
WSCALE = 32.0    # host weight pre-scale
OSCALE = 64.0    # attention-output scale (ones column = 1/OSCALE)


def _chunks(total, size):
    return [(o, min(size, total - o)) for o in range(0, total, size)]


def flags_for(inputs):
    def nz(*keys):
        return any(np.any(np.asarray(inputs[k])) for k in keys)
    return dict(
        bqk_g=nz("gbq", "gbk"), bqk_l=nz("lbq", "lbk"),
        bv_g=nz("gbv"), bv_l=nz("lbv"),
        ln=(not np.all(np.asarray(inputs["gamma"]) == 1.0)
            or np.any(np.asarray(inputs["beta"]))),
    )


DEFAULT_FLAGS = dict(bqk_g=False, bqk_l=False, bv_g=False, bv_l=False,
                     ln=False)


def build_nc(cfg=None, flags=None):
    """Build + compile the per-core Bass program (SPMD, same on all cores)."""
    import concourse.bass as bass
    import concourse.tile as tile
    import concourse.mybir as mybir
    from concourse import bacc

    cfg = dict(cfg or FULL_CFG)
    flags = dict(flags or DEFAULT_FLAGS)
    S, D, H, K, NW = cfg["S"], cfg["D"], cfg["H"], cfg["K"], cfg["NW"]
    HK = H * K
    SH = S // 2          # per-core query rows (half the sequence)
    WIN = S // NW        # local attention window
    NWH = SH // WIN      # windows in this core's half
    assert K == 64 and D % 256 == 0 and HK % 128 == 0
    assert SH % 128 == 0 and WIN % 128 == 0 and NWH * WIN == SH

    ND2 = D // 256       # d-pair tiles (DoubleRow contracts 256 at a time)
    NHK = HK // 128      # head-pair tiles (2 heads each)
    NST = S // 128       # s-tiles (full seq)
    NSP = NST // 2       # s-tile pairs (global)
    NQT = SH // 128      # q-tiles (half seq)
    NLP = SH // 256      # local s-tile pairs = windows in half
    NOT = NHK // 2       # hk-pair tiles for output proj (4)

    f32 = mybir.dt.float32
    fp8 = mybir.dt.float8e4
    DR = mybir.MatmulPerfMode.DoubleRow
    Exp = mybir.ActivationFunctionType.Exp
    Square = mybir.ActivationFunctionType.Square
    Sqrt = mybir.ActivationFunctionType.Sqrt
    add_op = mybir.AluOpType.add
    mult_op = mybir.AluOpType.mult
    sub_op = mybir.AluOpType.subtract
    AxX = mybir.AxisListType.X

    nc = bacc.Bacc("TRN2", target_bir_lowering=False, debug=False,
                   num_devices=N_CORES)

    # ---- DRAM parameters -------------------------------------------------
    xT2_d = nc.dram_tensor("xT2", [ND2, 128, 2, S], fp8, kind="ExternalInput")
    xq_d = nc.dram_tensor("xq", [SH, D], f32, kind="ExternalInput")
    w_d = {}
    for nm in ("wq_g", "wk_g", "wv_g", "wq_l", "wk_l", "wv_l"):
        w_d[nm] = nc.dram_tensor(nm, [ND2, 128, 2, HK], fp8,
                                 kind="ExternalInput")
    wo_d = {st: nc.dram_tensor(f"wo_{st}", [NOT, 128, 2, D], fp8,
                               kind="ExternalInput") for st in ("g", "l")}
    bcol_d = {}
    for st in ("g", "l"):
        if flags[f"bqk_{st}"]:
            for qk in ("q", "k"):
                bcol_d[f"b{qk}_{st}"] = nc.dram_tensor(
                    f"b{qk}_{st}", [NHK, 128], f32, kind="ExternalInput")
        if flags[f"bv_{st}"]:
            bcol_d[f"bv_{st}"] = nc.dram_tensor(
                f"bv_{st}", [1, HK], fp8, kind="ExternalInput")
    if flags["ln"]:
        gamma_d = nc.dram_tensor("gamma", [1, D], f32, kind="ExternalInput")
        beta_d = nc.dram_tensor("beta", [1, D], f32, kind="ExternalInput")
    out_d = nc.dram_tensor("out", [SH, D], f32, kind="ExternalOutput")
    taps = {}
    if flags.get("dbg"):
        taps["tap_k"] = nc.dram_tensor("tap_k", [128, S], fp8, kind="ExternalOutput")
        taps["tap_q"] = nc.dram_tensor("tap_q", [128, SH], fp8, kind="ExternalOutput")
        taps["tap_vx"] = nc.dram_tensor("tap_vx", [128, 2 * H * 128], fp8, kind="ExternalOutput")
        taps["tap_rs"] = nc.dram_tensor("tap_rs", [1, SH], f32, kind="ExternalOutput")
        taps["tap_ri"] = nc.dram_tensor("tap_ri", [1, SH], f32, kind="ExternalOutput")
        taps["tap_o8"] = nc.dram_tensor("tap_o8", [64, SH], fp8, kind="ExternalOutput")
        taps["tap_ex"] = nc.dram_tensor("tap_ex", [128, 2 * SH], fp8, kind="ExternalOutput")

    # DRAM scratch for attention outputs, laid out for the out-proj DoubleRow
    # load: [t, p, j, q] with hk = 256 t + 128 j + p
    oscr = {st: nc.dram_tensor(f"oscr_{st}", [NOT, 128, 2, SH], fp8)
            for st in ("g", "l")}

    PS = bass.MemorySpace.PSUM

    with tile.TileContext(nc) as tc, ExitStack() as ctx:
        # ---- constants ---------------------------------------------------
        cpool = ctx.enter_context(tc.tile_pool(name="consts", bufs=1))
        ones8 = cpool.tile([1, 128], fp8, tag="ones", name="ones")
        nc.vector.memset(ones8[:], 1.0)
        eps_col = cpool.tile([128, 1], f32, tag="eps", name="eps")
        nc.vector.memset(eps_col[:], float(LN_EPS))
        bcol_sb = {}
        for nm, d in bcol_d.items():
            if nm.startswith("bv"):
                t = cpool.tile([1, HK], fp8, tag=nm, name=nm)
                nc.sync.dma_start(t[:], d[:])
                bcol_sb[nm] = t
            else:
                cols = []
                for j in range(NHK):
                    t = cpool.tile([128, 1], f32, tag=f"{nm}{j}", name=f"{nm}{j}")
                    nc.sync.dma_start(t[:], d[j, :].rearrange("(a b) -> a b", b=1))
                    cols.append(t)
                bcol_sb[nm] = cols

        wopool = ctx.enter_context(tc.tile_pool(name="wo", bufs=1))

        # ---- persistent K/Q/V tiles (released before phase D) ------------
        mid_ctx = ctx.enter_context(ExitStack())
        kqv = mid_ctx.enter_context(tc.tile_pool(name="kqv", bufs=1))
        kT_g = [kqv.tile([128, S], fp8, tag=f"ktg{j}", name=f"ktg{j}") for j in range(NHK)]
        qT_g = [kqv.tile([128, SH], fp8, tag=f"qtg{j}", name=f"qtg{j}") for j in range(NHK)]
        vx_g = [kqv.tile([128, 2, H, 128], fp8, tag=f"vxg{u}", name=f"vxg{u}")
                for u in range(NSP)]
        kT_l = [kqv.tile([128, SH], fp8, tag=f"ktl{j}", name=f"ktl{j}") for j in range(NHK)]
        qT_l = [kqv.tile([128, SH], fp8, tag=f"qtl{j}", name=f"qtl{j}") for j in range(NHK)]
        vx_l = [kqv.tile([128, 2, H, 128], fp8, tag=f"vxl{u}", name=f"vxl{u}")
                for u in range(NLP)]
        for u in range(NSP):
            nc.vector.memset(vx_g[u][:, :, :, 0:64], 1.0 / OSCALE)
        for u in range(NLP):
            nc.vector.memset(vx_l[u][:, :, :, 0:64], 1.0 / OSCALE)

        # ---- x^T tiles (persist through local projections) ---------------
        xin = mid_ctx.enter_context(tc.tile_pool(name="xin", bufs=1))
        x2t = [xin.tile([128, 2, S], fp8, tag=f"xt{t}", name=f"xt{t}") for t in range(ND2)]
        for t in range(ND2):
            nc.sync.dma_start(x2t[t][:], xT2_d[t])

        # ---- weights (all six projection sets + wo, loaded up front) ------
        wpool = mid_ctx.enter_context(tc.tile_pool(name="wt", bufs=1))
        w_sb = {}
        for nm in ("wk_g", "wq_g", "wv_g", "wk_l", "wq_l", "wv_l"):
            w_sb[nm] = [wpool.tile([128, 2, HK], fp8, tag=f"{nm}{t}", name=f"{nm}{t}")
                        for t in range(ND2)]
            for t in range(ND2):
                nc.sync.dma_start(w_sb[nm][t][:], w_d[nm][t])
        wo_sb = {}
        for st in ("g", "l"):
            wo_sb[st] = [wopool.tile([128, 2, D], fp8, tag=f"wo{st}{t}", name=f"wo{st}{t}")
                         for t in range(NOT)]
            for t in range(NOT):
                nc.sync.dma_start(wo_sb[st][t][:], wo_d[st][t])

        # ================= Phase A: global projections ====================
        # proj_kq writes out^T tiles [128 hk, s]; proj_v writes vx tiles.
        pA = tc.tile_pool(name="ppA", bufs=2, space=PS)
        with pA as ppA:
            def proj_kq_g(nm, s_len, out_tiles, bias):
                for j in range(NHK):
                    pt = ppA.tile([128, 2048], f32, tag="pp", name=f"p{nm}{j}")
                    for t in range(ND2):
                        for so, sl in _chunks(s_len, 512):
                            nc.tensor.matmul(
                                pt[:, so:so + sl],
                                w_sb[nm][t][:, :, j * 128:(j + 1) * 128],
                                x2t[t][:, :, so:so + sl],
                                start=(t == 0), stop=(t == ND2 - 1),
                                perf_mode=DR)
                    if bias is not None:
                        nc.vector.tensor_scalar(
                            out_tiles[j][:], pt[:, 0:s_len], 1.0 / WSCALE,
                            bias[j], mult_op, add_op)
                    else:
                        nc.vector.tensor_scalar(
                            out_tiles[j][:], pt[:, 0:s_len], 1.0 / WSCALE,
                            None, mult_op)

            def proj_v_g(nm, s_len, vx_tiles, bias_row):
                # two s-tiles share one psum tile (pair layout for DR AV)
                for u in range(s_len // 256):
                    pt = ppA.tile([128, 2048], f32, tag="pp", name=f"pv{u}")
                    for jj in range(2):
                        ts_ = 2 * u + jj
                        for t in range(ND2):
                            for ho, hl in _chunks(HK, 512):
                                st_ = t == 0
                                sp_ = t == ND2 - 1 and bias_row is None
                                nc.tensor.matmul(
                                    pt[:, jj * 1024 + ho:jj * 1024 + ho + hl],
                                    x2t[t][:, :, ts_ * 128:(ts_ + 1) * 128],
                                    w_sb[nm][t][:, :, ho:ho + hl],
                                    start=st_, stop=sp_, perf_mode=DR)
                        if bias_row is not None:
                            for ho, hl in _chunks(HK, 512):
                                nc.tensor.matmul(
                                    pt[:, jj * 1024 + ho:jj * 1024 + ho + hl],
                                    ones8[0:1, 0:128], bias_row[0:1, ho:ho + hl],
                                    start=False, stop=True)
                        nc.vector.tensor_scalar(
                            vx_tiles[u][:, jj, :, 64:128],
                            pt[:, jj * 1024:(jj + 1) * 1024].rearrange(
                                "p (h k) -> p h k", k=64),
                            1.0 / WSCALE, None, mult_op)

            proj_kq_g("wk_g", S, kT_g, bcol_sb.get("bk_g"))
            proj_kq_g("wq_g", SH, qT_g, bcol_sb.get("bq_g"))
            proj_v_g("wv_g", S, vx_g, bcol_sb.get("bv_g"))
            if flags.get("dbg"):
                nc.sync.dma_start(taps["tap_k"][:], kT_g[0][:])
                nc.sync.dma_start(taps["tap_q"][:], qT_g[0][:])
                nc.sync.dma_start(taps["tap_vx"][:],
                                  vx_g[0][:].rearrange("p a h k -> p (a h k)"))

        # ============== Phases B/C: attention + local projections =========
        bc_ctx = ctx.enter_context(ExitStack())
        scp = bc_ctx.enter_context(tc.tile_pool(name="scp", bufs=2, space=PS))
        opp = bc_ctx.enter_context(tc.tile_pool(name="opp", bufs=1, space=PS))
        ppB = bc_ctx.enter_context(tc.tile_pool(name="ppB", bufs=2, space=PS))
        exp_p = bc_ctx.enter_context(tc.tile_pool(name="exp", bufs=3))
        nop = bc_ctx.enter_context(tc.tile_pool(name="nop", bufs=2))

        # --- local projection fill work (emitted between global heads) ----
        def fill_kq_l(nm, out_tiles, bias, j):
            for so, sl in _chunks(SH, 512):
                pt = ppB.tile([128, 512], f32, tag="ppb", name=f"f{nm}{j}{so}")
                for t in range(ND2):
                    nc.tensor.matmul(
                        pt[:], w_sb[nm][t][:, :, j * 128:(j + 1) * 128],
                        x2t[t][:, :, so:so + sl],
                        start=(t == 0), stop=(t == ND2 - 1), perf_mode=DR)
                if bias is not None:
                    nc.vector.tensor_scalar(
                        out_tiles[j][:, so:so + sl], pt[:], 1.0 / WSCALE,
                        bias[j], mult_op, add_op)
                else:
                    nc.vector.tensor_scalar(
                        out_tiles[j][:, so:so + sl], pt[:], 1.0 / WSCALE,
                        None, mult_op)

        def fill_v_l(nm, vx_tiles, bias_row, ts_):
            u, jj = divmod(ts_, 2)
            for ho, hl in _chunks(HK, 512):
                pt = ppB.tile([128, 512], f32, tag="ppb", name=f"fv{ts_}{ho}")
                for t in range(ND2):
                    st_ = t == 0
                    sp_ = t == ND2 - 1 and bias_row is None
                    nc.tensor.matmul(
                        pt[:], x2t[t][:, :, ts_ * 128:(ts_ + 1) * 128],
                        w_sb[nm][t][:, :, ho:ho + hl],
                        start=st_, stop=sp_, perf_mode=DR)
                if bias_row is not None:
                    nc.tensor.matmul(pt[:], ones8[0:1, 0:128],
                                     bias_row[0:1, ho:ho + hl],
                                     start=False, stop=True)
                nc.vector.tensor_scalar(
                    vx_tiles[u][:, jj, ho // 64:(ho + hl) // 64, 64:128],
                    pt[:].rearrange("p (h k) -> p h k", k=64),
                    1.0 / WSCALE, None, mult_op)

        fills = deque()
        for j in range(NHK):
            fills.append(lambda j=j: fill_kq_l(
                "wk_l", kT_l, bcol_sb.get("bk_l"), j))
        for j in range(NHK):
            fills.append(lambda j=j: fill_kq_l(
                "wq_l", qT_l, bcol_sb.get("bq_l"), j))
        for ts_ in range(SH // 128):
            fills.append(lambda ts_=ts_: fill_v_l(
                "wv_l", vx_l, bcol_sb.get("bv_l"), ts_))

        def normalize_store(h, o_ps, dst, dbg=False):
            # rinv = OSCALE / rowsum ; o8 = o * rinv  (x OSCALE into fp8)
            rinv = nop.tile([1, SH], f32, tag="ri", name=f"ri{h}")
            nc.vector.reciprocal_approx_fast(out=rinv[:], in_=o_ps[0:1, :])
            rb = nop.tile([64, SH], f32, tag="rb", name=f"rb{h}")
            nc.gpsimd.partition_broadcast(rb[:], rinv[0:1, :])
            o8 = nop.tile([64, SH], fp8, tag="o8", name=f"o8{h}")
            nc.vector.tensor_tensor(o8[:], o_ps[64:128, :], rb[:], mult_op)
            if dbg:
                rs = nop.tile([1, SH], f32, tag="rs", name="rs")
                nc.vector.tensor_copy(rs[:], o_ps[0:1, :])
                nc.sync.dma_start(taps["tap_rs"][:], rs[:])
                nc.sync.dma_start(taps["tap_ri"][:], rinv[:])
                nc.sync.dma_start(taps["tap_o8"][:], o8[:])
            t, j, pr = h // 4, (h % 4) // 2, 64 * (h % 2)
            nc.sync.dma_start(dst[t][pr:pr + 64, j, :], o8[:])

        # --- global attention, one head at a time -------------------------
        for h in range(H):
            hp, po = h // 2, 64 * (h % 2)
            o_ps = opp.tile([128, SH], f32, tag="o", name=f"og{h}")
            pend = deque()

            def do_av(item):
                ex2, u = item
                for qo, ql in _chunks(SH, 512):
                    nc.tensor.matmul(
                        o_ps[:, qo:qo + ql], vx_g[u][:, :, h, :],
                        ex2[:, :, qo:qo + ql],
                        start=(u == 0), stop=(u == NSP - 1), perf_mode=DR)

            for u in range(NSP):
                ex2 = exp_p.tile([128, 2, SH], fp8, tag="ex", name=f"exg{h}{u}")
                for j in range(2):
                    st = 2 * u + j
                    sc = scp.tile([128, SH], f32, tag="sc", name=f"scg{h}{st}")
                    for qo, ql in _chunks(SH, 512):
                        nc.tensor.matmul(
                            sc[:, qo:qo + ql],
                            kT_g[hp][po:po + 64, st * 128:(st + 1) * 128],
                            qT_g[hp][po:po + 64, qo:qo + ql],
                            start=True, stop=True)
                    nc.scalar.activation(ex2[:, j, :], sc[:], Exp, scale=0.125)
                if flags.get("dbg") and h == 0 and u == 0:
                    nc.sync.dma_start(taps["tap_ex"][:],
                                      ex2[:].rearrange("p a b -> p (a b)"))
                pend.append((ex2, u))
                if len(pend) > 1:
                    do_av(pend.popleft())
            while pend:
                do_av(pend.popleft())
            normalize_store(h, o_ps, oscr["g"], dbg=flags.get("dbg") and h == 0)

            # interleave local projection work into the scalar-bound gaps
            for _ in range(2):
                if fills:
                    fills.popleft()()

        while fills:
            fills.popleft()()

        # --- local attention ----------------------------------------------
        for h in range(H):
            hp, po = h // 2, 64 * (h % 2)
            o_ps = opp.tile([128, SH], f32, tag="o", name=f"ol{h}")
            ex2 = exp_p.tile([128, 2, SH], fp8, tag="ex", name=f"exl{h}")
            for ss in range(2):
                sc = scp.tile([128, SH], f32, tag="sc", name=f"scl{h}{ss}")
                for w in range(NWH):
                    st = 2 * w + ss
                    nc.tensor.matmul(
                        sc[:, w * WIN:(w + 1) * WIN],
                        kT_l[hp][po:po + 64, st * 128:(st + 1) * 128],
                        qT_l[hp][po:po + 64, w * WIN:(w + 1) * WIN],
                        start=True, stop=True)
                nc.scalar.activation(ex2[:, ss, :], sc[:], Exp, scale=0.125)
            for w in range(NWH):
                nc.tensor.matmul(
                    o_ps[:, w * WIN:(w + 1) * WIN], vx_l[w][:, :, h, :],
                    ex2[:, :, w * WIN:(w + 1) * WIN],
                    start=(w % 2 == 0), stop=(w % 2 == 1), perf_mode=DR)
            normalize_store(h, o_ps, oscr["l"])

        # ========== Phase D: output projection + residual + layernorm ====
        bc_ctx.close()   # release attention PSUM banks for the ypp pool
        mid_ctx.close()  # release K/Q/V + x + projection-weight SBUF

        with tc.tile_pool(name="od", bufs=1) as odp, \
             tc.tile_pool(name="ypp", bufs=2, space=PS) as ypp, \
             tc.tile_pool(name="ln", bufs=2) as lnp:
            if flags["ln"]:
                gamma_bc = lnp.tile([128, D], f32, tag="gamma", name="gamma", bufs=1)
                nc.sync.dma_start(gamma_bc[:], gamma_d[:].partition_broadcast(128))
                beta_bc = lnp.tile([128, D], f32, tag="beta", name="beta", bufs=1)
                nc.sync.dma_start(beta_bc[:], beta_d[:].partition_broadcast(128))
            o_sb = {}
            for st in ("g", "l"):
                o_sb[st] = [odp.tile([128, 2, SH], fp8, tag=f"ob{st}{t}", name=f"ob{st}{t}")
                            for t in range(NOT)]
                for t in range(NOT):
                    nc.sync.dma_start(o_sb[st][t][:], oscr[st][t])
            for qt in range(NQT):
                ps_y = ypp.tile([128, D], f32, tag="py", name=f"py{qt}")
                for do, dl in _chunks(D, 512):
                    first = True
                    for st in ("g", "l"):
                        for t in range(NOT):
                            nc.tensor.matmul(
                                ps_y[:, do:do + dl],
                                o_sb[st][t][:, :, qt * 128:(qt + 1) * 128],
                                wo_sb[st][t][:, :, do:do + dl],
                                start=first, stop=(st == "l" and t == NOT - 1),
                                perf_mode=DR)
                            first = False
                xq_t = lnp.tile([128, D], f32, tag="xq", name=f"xq{qt}")
                nc.sync.dma_start(xq_t[:], xq_d[qt * 128:(qt + 1) * 128, :])
                y = lnp.tile([128, D], f32, tag="y", name=f"y{qt}")
                ssum = lnp.tile([128, 1], f32, tag="ssum", name=f"ssum{qt}")
                nc.vector.scalar_tensor_tensor(
                    y[:], ps_y[:], 1.0 / (WSCALE * OSCALE), xq_t[:],
                    mult_op, add_op, accum_out=ssum[:])
                sqd = lnp.tile([128, D], f32, tag="sqd", name=f"sqd{qt}")
                ssq = lnp.tile([128, 1], f32, tag="ssq", name=f"ssq{qt}")
                nc.scalar.activation(sqd[:], y[:], Square, accum_out=ssq[:])
                mu = lnp.tile([128, 1], f32, tag="mu", name=f"mu{qt}")
                nc.vector.tensor_scalar_mul(mu[:], ssum[:], 1.0 / D)
                var = lnp.tile([128, 1], f32, tag="var", name=f"var{qt}")
                nc.vector.tensor_scalar_mul(var[:], ssq[:], 1.0 / D)
                mu2 = lnp.tile([128, 1], f32, tag="mu2", name=f"mu2{qt}")
                nc.vector.tensor_tensor(mu2[:], mu[:], mu[:], mult_op)
                nc.vector.tensor_tensor(var[:], var[:], mu2[:], sub_op)
                sd = lnp.tile([128, 1], f32, tag="sd", name=f"sd{qt}")
                nc.scalar.activation(sd[:], var[:], Sqrt, bias=eps_col[:])
                rstd = lnp.tile([128, 1], f32, tag="rstd", name=f"rstd{qt}")
                nc.vector.reciprocal(rstd[:], sd[:])
                bco = lnp.tile([128, 1], f32, tag="bco", name=f"bco{qt}")
                nc.vector.tensor_tensor(bco[:], mu[:], rstd[:], mult_op)
                nc.vector.tensor_scalar_mul(bco[:], bco[:], -1.0)
                ot = lnp.tile([128, D], f32, tag="ot", name=f"ot{qt}")
                nc.vector.tensor_scalar(ot[:], y[:], rstd[:], bco[:],
                                        mult_op, add_op)
                if flags["ln"]:
                    t2 = lnp.tile([128, D], f32, tag="t2", name=f"t2{qt}")
                    nc.vector.tensor_tensor(t2[:], ot[:], gamma_bc[:], mult_op)
                    nc.vector.tensor_tensor(ot[:], t2[:], beta_bc[:], add_op)
                nc.sync.dma_start(out_d[qt * 128:(qt + 1) * 128, :], ot[:])

    nc.compile()
    return nc


def make_in_maps(inputs, cfg=None, flags=None):
    """Build per-core input maps from the full (unsharded) problem inputs."""
    cfg = dict(cfg or FULL_CFG)
    flags = dict(flags or DEFAULT_FLAGS)
    S, D, H, K = cfg["S"], cfg["D"], cfg["H"], cfg["K"]
    HK = H * K
    SH = S // 2
    ND2 = D // 256
    NHK = HK // 128
    NOT = NHK // 2

    def np32(a):
        return np.asarray(a, dtype=np.float32)

    def dpair(w):  # [D, X] -> [ND2, 128, 2, X]
        return np.ascontiguousarray(
            w.reshape(ND2, 2, 128, -1).transpose(0, 2, 1, 3))

    shared = {}
    for nm, key in (("wq_g", "gWq"), ("wk_g", "gWk"), ("wv_g", "gWv"),
                    ("wq_l", "lWq"), ("wk_l", "lWk"), ("wv_l", "lWv")):
        shared[nm] = dpair(np32(inputs[key]).reshape(D, HK) * WSCALE).astype(F8)
    for st, key in (("g", "gWo"), ("l", "lWo")):
        w = np32(inputs[key]).reshape(HK, D) * WSCALE
        shared[f"wo_{st}"] = np.ascontiguousarray(
            w.reshape(NOT, 2, 128, D).transpose(0, 2, 1, 3)).astype(F8)
    for st, q, k in (("g", "gbq", "gbk"), ("l", "lbq", "lbk")):
        if flags[f"bqk_{st}"]:
            shared[f"bq_{st}"] = np.ascontiguousarray(
                np32(inputs[q]).reshape(NHK, 128))
            shared[f"bk_{st}"] = np.ascontiguousarray(
                np32(inputs[k]).reshape(NHK, 128))
    for st, key in (("g", "gbv"), ("l", "lbv")):
        if flags[f"bv_{st}"]:
            shared[f"bv_{st}"] = (np32(inputs[key]).reshape(1, HK)
                                  * WSCALE).astype(F8)
    if flags["ln"]:
        shared["gamma"] = np32(inputs["gamma"]).reshape(1, D)
        shared["beta"] = np32(inputs["beta"]).reshape(1, D)

    x = np32(inputs["x"])
    bo = np32(inputs["gbo"]) + np32(inputs["lbo"])
    in_maps = []
    for c in range(N_CORES):
        b, half = divmod(c, 2)
        xb = x[b]
        # own half first (queries/local), other half second; global attention
        # is invariant to key/value column order
        xperm = np.concatenate([xb[half * SH:(half + 1) * SH],
                                xb[(1 - half) * SH:(2 - half) * SH]], axis=0)
        m = dict(shared)
        m["xT2"] = dpair(np.ascontiguousarray(xperm.T)).astype(F8)
        m["xq"] = np.ascontiguousarray(xperm[0:SH]) + bo
        in_maps.append(m)
    return in_maps


def assemble_out(results, cfg=None):
    cfg = dict(cfg or FULL_CFG)
    S, D = cfg["S"], cfg["D"]
    SH = S // 2
    B = N_CORES // 2
    out = np.empty((B, S, D), np.float32)
    for c in range(N_CORES):
        b, half = divmod(c, 2)
        out[b, half * SH:(half + 1) * SH] = results[c]["out"]
    return out


_NC_CACHE = {}


def kernel(**inputs):
    from concourse.bass_utils import run_bass_kernel_spmd
    flags = flags_for(inputs)
    key = tuple(sorted(flags.items()))
    if key not in _NC_CACHE:
        _NC_CACHE[key] = build_nc(flags=flags)
    nc = _NC_CACHE[key]
    in_maps = make_in_maps(inputs, flags=flags)
    res = run_bass_kernel_spmd(nc, in_maps, list(range(N_CORES)))
    return assemble_out(res.results)


# revision 16
# speedup vs baseline: 1.6979x; 1.0558x over previous
"""Trainium2 Bass kernel for LocalGlobalSelfAttention (fp8 DoubleRow).

Sharding: 8 cores = 4 batches x 2 sequence-halves (no collectives).
Each core computes, for its (batch b, half h):
  - global attention: queries = its half (SH rows), keys/values = full seq
  - local windowed attention: fully contained in its half
  - output projections (g+l accumulated in PSUM) + residual + layernorm

All matmuls run fp8e4 with fp32 PSUM accumulation; contraction-128 matmuls
(projections, AV, output projection) use DoubleRow (2x effective throughput).
Scores (K=64 contraction) run plain fp8. Scaling ledger: weights host-scaled
x32 (fp8 subnormal avoidance), un-scaled in the PSUM->SBUF cast; attention
outputs carry x64 (ones-column = 1/64, so the rowsum reciprocal yields
64/sum); the final combine divides by 64*32 = 2048. Softmax skips the max
subtraction (scores are O(1)); rowsums ride the AV matmul via the ones
column (placed FIRST so the rowsum lands in psum partition 0 - the custom
DVE reciprocal ignores AP partition offsets); v dims sit at columns 64-127
(64-partition reads must start at partition 0 or 64).

Pipeline: the global-attention phase is scalar(exp)-bound, so all deferrable
tensor work - v projections, the six local projection sets, local attention
heads, and out-proj operand DMA loads - is emitted as fill units between
score/exp pairs of the global heads (emission order matters: engine queues
are in-order FIFOs, so every dependency of an instruction must be emitted
before it).
"""

import numpy as np
import ml_dtypes
from collections import deque
from contextlib import ExitStack

F8 = ml_dtypes.float8_e4m3

FULL_CFG = dict(S=2048, D=1024, H=16, K=64, NW=8)
N_CORES = 8
LN_EPS = 1e-3
WSCALE = 32.0    # host weight pre-scale
OSCALE = 64.0    # attention-output scale (ones column = 1/OSCALE)


def _chunks(total, size):
    return [(o, min(size, total - o)) for o in range(0, total, size)]


def flags_for(inputs):
    def nz(*keys):
        return any(np.any(np.asarray(inputs[k])) for k in keys)
    return dict(
        bqk_g=nz("gbq", "gbk"), bqk_l=nz("lbq", "lbk"),
        bv_g=nz("gbv"), bv_l=nz("lbv"),
        ln=(not np.all(np.asarray(inputs["gamma"]) == 1.0)
            or np.any(np.asarray(inputs["beta"]))),
    )


DEFAULT_FLAGS = dict(bqk_g=False, bqk_l=False, bv_g=False, bv_l=False,
                     ln=False)


def build_nc(cfg=None, flags=None):
    """Build + compile the per-core Bass program (SPMD, same on all cores)."""
    import concourse.bass as bass
    import concourse.tile as tile
    import concourse.mybir as mybir
    from concourse import bacc

    cfg = dict(cfg or FULL_CFG)
    flags = dict(flags or DEFAULT_FLAGS)
    S, D, H, K, NW = cfg["S"], cfg["D"], cfg["H"], cfg["K"], cfg["NW"]
    HK = H * K
    SH = S // 2          # per-core query rows (half the sequence)
    WIN = S // NW        # local attention window
    NWH = SH // WIN      # windows in this core's half
    assert K == 64 and D % 256 == 0 and HK % 128 == 0
    assert SH % 128 == 0 and WIN % 128 == 0 and NWH * WIN == SH

    ND2 = D // 256       # d-pair tiles (DoubleRow contracts 256 at a time)
    NHK = HK // 128      # head-pair tiles (2 heads each)
    NST = S // 128       # s-tiles (full seq)
    NSP = NST // 2       # s-tile pairs (global)
    NQT = SH // 128      # q-tiles (half seq)
    NLP = SH // 256      # local s-tile pairs = windows in half
    NOT = NHK // 2       # hk-pair tiles for output proj

    f32 = mybir.dt.float32
    fp8 = mybir.dt.float8e4
    DR = mybir.MatmulPerfMode.DoubleRow
    Exp = mybir.ActivationFunctionType.Exp
    Square = mybir.ActivationFunctionType.Square
    Sqrt = mybir.ActivationFunctionType.Sqrt
    add_op = mybir.AluOpType.add
    mult_op = mybir.AluOpType.mult
    sub_op = mybir.AluOpType.subtract

    nc = bacc.Bacc("TRN2", target_bir_lowering=False, debug=False,
                   num_devices=N_CORES)

    # ---- DRAM parameters -------------------------------------------------
    xT2_d = nc.dram_tensor("xT2", [ND2, 128, 2, S], fp8, kind="ExternalInput")
    xq_d = nc.dram_tensor("xq", [SH, D], f32, kind="ExternalInput")
    w_d = {}
    for nm in ("wq_g", "wk_g", "wv_g", "wq_l", "wk_l", "wv_l"):
        w_d[nm] = nc.dram_tensor(nm, [ND2, 128, 2, HK], fp8,
                                 kind="ExternalInput")
    wo_d = {st: nc.dram_tensor(f"wo_{st}", [NOT, 128, 2, D], fp8,
                               kind="ExternalInput") for st in ("g", "l")}
    bcol_d = {}
    for st in ("g", "l"):
        if flags[f"bqk_{st}"]:
            for qk in ("q", "k"):
                bcol_d[f"b{qk}_{st}"] = nc.dram_tensor(
                    f"b{qk}_{st}", [NHK, 128], f32, kind="ExternalInput")
        if flags[f"bv_{st}"]:
            bcol_d[f"bv_{st}"] = nc.dram_tensor(
                f"bv_{st}", [1, HK], fp8, kind="ExternalInput")
    if flags["ln"]:
        gamma_d = nc.dram_tensor("gamma", [1, D], f32, kind="ExternalInput")
        beta_d = nc.dram_tensor("beta", [1, D], f32, kind="ExternalInput")
    out_d = nc.dram_tensor("out", [SH, D], f32, kind="ExternalOutput")

    # DRAM scratch for attention outputs, laid out for the out-proj DoubleRow
    # load: [t, p, j, q] with hk = 256 t + 128 j + p
    oscr = {st: nc.dram_tensor(f"oscr_{st}", [NOT, 128, 2, SH], fp8)
            for st in ("g", "l")}

    PS = bass.MemorySpace.PSUM

    with tile.TileContext(nc) as tc, ExitStack() as ctx:
        # ---- whole-life pools -------------------------------------------
        cpool = ctx.enter_context(tc.tile_pool(name="consts", bufs=1))
        wopool = ctx.enter_context(tc.tile_pool(name="wo", bufs=1))
        odpool = ctx.enter_context(tc.tile_pool(name="od", bufs=1))

        ones8 = cpool.tile([1, 128], fp8, tag="ones", name="ones")
        nc.vector.memset(ones8[:], 1.0)
        eps_col = cpool.tile([128, 1], f32, tag="eps", name="eps")
        nc.vector.memset(eps_col[:], float(LN_EPS))
        bcol_sb = {}
        for nm, d in bcol_d.items():
            if nm.startswith("bv"):
                t = cpool.tile([1, HK], fp8, tag=nm, name=nm)
                nc.sync.dma_start(t[:], d[:])
                bcol_sb[nm] = t
            else:
                cols = []
                for j in range(NHK):
                    t = cpool.tile([128, 1], f32, tag=f"{nm}{j}",
                                   name=f"{nm}{j}")
                    nc.sync.dma_start(t[:], d[j, :].rearrange("(a b) -> a b",
                                                              b=1))
                    cols.append(t)
                bcol_sb[nm] = cols

        wo_sb = {}
        for st in ("g", "l"):
            wo_sb[st] = [wopool.tile([128, 2, D], fp8, tag=f"wo{st}{t}",
                                     name=f"wo{st}{t}") for t in range(NOT)]
        o_sb = {st: [odpool.tile([128, 2, SH], fp8, tag=f"ob{st}{t}",
                                 name=f"ob{st}{t}") for t in range(NOT)]
                for st in ("g", "l")}

        # ---- mid-life pools (released before phase D) --------------------
        mid_ctx = ctx.enter_context(ExitStack())
        kqv = mid_ctx.enter_context(tc.tile_pool(name="kqv", bufs=1))
        xin = mid_ctx.enter_context(tc.tile_pool(name="xin", bufs=1))
        wpool = mid_ctx.enter_context(tc.tile_pool(name="wt", bufs=1))

        kT_g = [kqv.tile([128, S], fp8, tag=f"ktg{j}", name=f"ktg{j}")
                for j in range(NHK)]
        qT_g = [kqv.tile([128, SH], fp8, tag=f"qtg{j}", name=f"qtg{j}")
                for j in range(NHK)]
        vx_g = [kqv.tile([128, 2, H, 128], fp8, tag=f"vxg{u}", name=f"vxg{u}")
                for u in range(NSP)]
        kT_l = [kqv.tile([128, SH], fp8, tag=f"ktl{j}", name=f"ktl{j}")
                for j in range(NHK)]
        qT_l = [kqv.tile([128, SH], fp8, tag=f"qtl{j}", name=f"qtl{j}")
                for j in range(NHK)]
        vx_l = [kqv.tile([128, 2, H, 128], fp8, tag=f"vxl{u}", name=f"vxl{u}")
                for u in range(NLP)]
        for u in range(NSP):
            nc.vector.memset(vx_g[u][:, :, :, 0:64], 1.0 / OSCALE)
        for u in range(NLP):
            nc.vector.memset(vx_l[u][:, :, :, 0:64], 1.0 / OSCALE)

        x2t = [xin.tile([128, 2, S], fp8, tag=f"xt{t}", name=f"xt{t}")
               for t in range(ND2)]
        w_sb = {}
        for nm in ("wk_g", "wq_g", "wv_g", "wk_l", "wq_l", "wv_l"):
            w_sb[nm] = [wpool.tile([128, 2, HK], fp8, tag=f"{nm}{t}",
                                   name=f"{nm}{t}") for t in range(ND2)]
        # DMA order: x first, then weights in first-use order
        for t in range(ND2):
            nc.sync.dma_start(x2t[t][:], xT2_d[t])
        for nm in ("wk_g", "wq_g", "wv_g", "wk_l", "wq_l", "wv_l"):
            for t in range(ND2):
                nc.sync.dma_start(w_sb[nm][t][:], w_d[nm][t])
        for st in ("g", "l"):
            for t in range(NOT):
                nc.sync.dma_start(wo_sb[st][t][:], wo_d[st][t])

        # ================= Phase A: global K/Q projections ================
        with tc.tile_pool(name="ppA", bufs=2, space=PS) as ppA:
            def proj_kq_g(nm, s_len, out_tiles, bias):
                for j in range(NHK):
                    pt = ppA.tile([128, 2048], f32, tag="pp", name=f"p{nm}{j}")
                    for t in range(ND2):
                        for so, sl in _chunks(s_len, 512):
                            nc.tensor.matmul(
                                pt[:, so:so + sl],
                                w_sb[nm][t][:, :, j * 128:(j + 1) * 128],
                                x2t[t][:, :, so:so + sl],
                                start=(t == 0), stop=(t == ND2 - 1),
                                perf_mode=DR)
                    if bias is not None:
                        nc.vector.tensor_scalar(
                            out_tiles[j][:], pt[:, 0:s_len], 1.0 / WSCALE,
                            bias[j], mult_op, add_op)
                    else:
                        nc.vector.tensor_scalar(
                            out_tiles[j][:], pt[:, 0:s_len], 1.0 / WSCALE,
                            None, mult_op)

            proj_kq_g("wk_g", S, kT_g, bcol_sb.get("bk_g"))
            proj_kq_g("wq_g", SH, qT_g, bcol_sb.get("bq_g"))

        # ============== Phases B/C: attention + deferred fill work ========
        bc_ctx = ctx.enter_context(ExitStack())
        scp = bc_ctx.enter_context(tc.tile_pool(name="scp", bufs=2, space=PS))
        opp = bc_ctx.enter_context(tc.tile_pool(name="opp", bufs=2, space=PS))
        exp_p = bc_ctx.enter_context(tc.tile_pool(name="exp", bufs=3))
        nop = bc_ctx.enter_context(tc.tile_pool(name="nop", bufs=2))

        # --- fill units: projections into scp-pool psum tiles -------------
        def fill_kq(nm, out_tiles, bias, j):
            pt = scp.tile([128, SH], f32, tag="sc", name=f"f{nm}{j}")
            for t in range(ND2):
                for so, sl in _chunks(SH, 512):
                    nc.tensor.matmul(
                        pt[:, so:so + sl],
                        w_sb[nm][t][:, :, j * 128:(j + 1) * 128],
                        x2t[t][:, :, so:so + sl],
                        start=(t == 0), stop=(t == ND2 - 1), perf_mode=DR)
            if bias is not None:
                nc.vector.tensor_scalar(out_tiles[j][:], pt[:], 1.0 / WSCALE,
                                        bias[j], mult_op, add_op)
            else:
                nc.vector.tensor_scalar(out_tiles[j][:], pt[:], 1.0 / WSCALE,
                                        None, mult_op)

        def fill_v(nm, vx_tiles, bias_row, ts_):
            u, jj = divmod(ts_, 2)
            pt = scp.tile([128, SH], f32, tag="sc", name=f"f{nm}{ts_}")
            for t in range(ND2):
                for ho, hl in _chunks(HK, 512):
                    st_ = t == 0
                    sp_ = t == ND2 - 1 and bias_row is None
                    nc.tensor.matmul(
                        pt[:, ho:ho + hl],
                        x2t[t][:, :, ts_ * 128:(ts_ + 1) * 128],
                        w_sb[nm][t][:, :, ho:ho + hl],
                        start=st_, stop=sp_, perf_mode=DR)
            if bias_row is not None:
                for ho, hl in _chunks(HK, 512):
                    nc.tensor.matmul(pt[:, ho:ho + hl], ones8[0:1, 0:128],
                                     bias_row[0:1, ho:ho + hl],
                                     start=False, stop=True)
            nc.vector.tensor_scalar(
                vx_tiles[u][:, jj, :, 64:128],
                pt[:].rearrange("p (h k) -> p h k", k=64),
                1.0 / WSCALE, None, mult_op)

        def normalize_store(h, o_ps, dst):
            # rinv = OSCALE / rowsum ; o8 = o * rinv  (x OSCALE into fp8)
            rinv = nop.tile([1, SH], f32, tag="ri", name=f"ri{h}", bufs=1)
            nc.vector.reciprocal_approx_fast(out=rinv[:], in_=o_ps[0:1, :])
            rb = nop.tile([64, SH], f32, tag="rb", name=f"rb{h}")
            nc.gpsimd.partition_broadcast(rb[:], rinv[0:1, :])
            o8 = nop.tile([64, SH], fp8, tag="o8", name=f"o8{h}")
            nc.vector.tensor_tensor(o8[:], o_ps[64:128, :], rb[:], mult_op)
            t, j, pr = h // 4, (h % 4) // 2, 64 * (h % 2)
            nc.sync.dma_start(dst[t][pr:pr + 64, j, :], o8[:])

        def local_head(h):
            hp, po = h // 2, 64 * (h % 2)
            o_ps = opp.tile([128, SH], f32, tag="o", name=f"ol{h}")
            ex2 = exp_p.tile([128, 2, SH], fp8, tag="ex", name=f"exl{h}")
            for ss in range(2):
                sc = scp.tile([128, SH], f32, tag="sc", name=f"scl{h}{ss}")
                for w in range(NWH):
                    st = 2 * w + ss
                    nc.tensor.matmul(
                        sc[:, w * WIN:(w + 1) * WIN],
                        kT_l[hp][po:po + 64, st * 128:(st + 1) * 128],
                        qT_l[hp][po:po + 64, w * WIN:(w + 1) * WIN],
                        start=True, stop=True)
                nc.scalar.activation(ex2[:, ss, :], sc[:], Exp, scale=0.125)
            for w in range(NWH):
                nc.tensor.matmul(
                    o_ps[:, w * WIN:(w + 1) * WIN], vx_l[w][:, :, h, :],
                    ex2[:, :, w * WIN:(w + 1) * WIN],
                    start=(w % 2 == 0), stop=(w % 2 == 1), perf_mode=DR)
            normalize_store(h, o_ps, oscr["l"])

        def load_o2(st, t):
            nc.sync.dma_start(o_sb[st][t][:], oscr[st][t])

        # fill queue: (cost_estimate_us, closure); emitted between score/exp
        # pairs. Order respects the in-order engine FIFOs: everything an
        # emitted instruction depends on is emitted earlier. v_g fills are
        # kept separate: ensure_vg() guarantees the producer of vx_g[u] is
        # emitted before any AV matmul that reads it.
        vg_next = [0]

        def ensure_vg(up_to_ts):
            while vg_next[0] <= min(up_to_ts, NST - 1):
                fill_v("wv_g", vx_g, bcol_sb.get("bv_g"), vg_next[0])
                vg_next[0] += 1

        fills = deque()
        for j in range(NHK):
            fills.append((1.8, lambda j=j: fill_kq(
                "wk_l", kT_l, bcol_sb.get("bk_l"), j)))
        for j in range(NHK):
            fills.append((1.8, lambda j=j: fill_kq(
                "wq_l", qT_l, bcol_sb.get("bq_l"), j)))
        for ts_ in range(SH // 128):
            fills.append((1.8, lambda ts_=ts_: fill_v(
                "wv_l", vx_l, bcol_sb.get("bv_l"), ts_)))
        for h in range(H):
            fills.append((2.2, lambda h=h: local_head(h)))
            if h % 4 == 3:
                fills.append((0.0, lambda t=h // 4: load_o2("l", t)))

        def pop_fills(budget):
            while fills and budget > 0:
                cost, fn = fills.popleft()
                fn()
                budget -= max(cost, 0.1)

        # v_g pairs 0-1 must exist before head 0's first AV matmuls
        ensure_vg(3)

        # --- global attention, one head at a time -------------------------
        for h in range(H):
            hp, po = h // 2, 64 * (h % 2)
            o_ps = opp.tile([128, SH], f32, tag="o", name=f"og{h}")
            pend = deque()

            def do_av(item, o_ps=o_ps, h=h):
                ex2, u = item
                ensure_vg(2 * u + 1)
                for qo, ql in _chunks(SH, 512):
                    nc.tensor.matmul(
                        o_ps[:, qo:qo + ql], vx_g[u][:, :, h, :],
                        ex2[:, :, qo:qo + ql],
                        start=(u == 0), stop=(u == NSP - 1), perf_mode=DR)

            for u in range(NSP):
                ex2 = exp_p.tile([128, 2, SH], fp8, tag="ex",
                                 name=f"exg{h}{u}")
                for j in range(2):
                    st = 2 * u + j
                    sc = scp.tile([128, SH], f32, tag="sc", name=f"scg{h}{st}")
                    for qo, ql in _chunks(SH, 512):
                        nc.tensor.matmul(
                            sc[:, qo:qo + ql],
                            kT_g[hp][po:po + 64, st * 128:(st + 1) * 128],
                            qT_g[hp][po:po + 64, qo:qo + ql],
                            start=True, stop=True)
                    nc.scalar.activation(ex2[:, j, :], sc[:], Exp, scale=0.125)
                pend.append((ex2, u))
                if len(pend) > 1:
                    do_av(pend.popleft())
                pop_fills(1.0)
            while pend:
                do_av(pend.popleft())
            ensure_vg(NST - 1)
            normalize_store(h, o_ps, oscr["g"])
            if h % 4 == 3:
                load_o2("g", h // 4)

        while fills:
            cost, fn = fills.popleft()
            fn()

        # ========== Phase D: output projection + residual + layernorm ====
        bc_ctx.close()   # release attention PSUM banks for the ypp pool
        mid_ctx.close()  # release K/Q/V + x + projection-weight SBUF

        with tc.tile_pool(name="ypp", bufs=2, space=PS) as ypp, \
             tc.tile_pool(name="ln", bufs=2) as lnp:
            if flags["ln"]:
                gamma_bc = lnp.tile([128, D], f32, tag="gamma", name="gamma",
                                    bufs=1)
                nc.sync.dma_start(gamma_bc[:],
                                  gamma_d[:].partition_broadcast(128))
                beta_bc = lnp.tile([128, D], f32, tag="beta", name="beta",
                                   bufs=1)
                nc.sync.dma_start(beta_bc[:],
                                  beta_d[:].partition_broadcast(128))
            for qt in range(NQT):
                xq_t = lnp.tile([128, D], f32, tag="xq", name=f"xq{qt}")
                nc.sync.dma_start(xq_t[:], xq_d[qt * 128:(qt + 1) * 128, :])
                ps_y = ypp.tile([128, D], f32, tag="py", name=f"py{qt}")
                for do, dl in _chunks(D, 512):
                    first = True
                    for st in ("g", "l"):
                        for t in range(NOT):
                            nc.tensor.matmul(
                                ps_y[:, do:do + dl],
                                o_sb[st][t][:, :, qt * 128:(qt + 1) * 128],
                                wo_sb[st][t][:, :, do:do + dl],
                                start=first, stop=(st == "l" and t == NOT - 1),
                                perf_mode=DR)
                            first = False
                y = lnp.tile([128, D], f32, tag="y", name=f"y{qt}")
                ssum = lnp.tile([128, 1], f32, tag="ssum", name=f"ssum{qt}")
                nc.vector.scalar_tensor_tensor(
                    y[:], ps_y[:], 1.0 / (WSCALE * OSCALE), xq_t[:],
                    mult_op, add_op, accum_out=ssum[:])
                sqd = lnp.tile([128, D], f32, tag="sqd", name=f"sqd{qt}")
                ssq = lnp.tile([128, 1], f32, tag="ssq", name=f"ssq{qt}")
                nc.scalar.activation(sqd[:], y[:], Square, accum_out=ssq[:])
                mu = lnp.tile([128, 1], f32, tag="mu", name=f"mu{qt}")
                nc.vector.tensor_scalar_mul(mu[:], ssum[:], 1.0 / D)
                var = lnp.tile([128, 1], f32, tag="var", name=f"var{qt}")
                nc.vector.tensor_scalar_mul(var[:], ssq[:], 1.0 / D)
                mu2 = lnp.tile([128, 1], f32, tag="mu2", name=f"mu2{qt}")
                nc.vector.tensor_tensor(mu2[:], mu[:], mu[:], mult_op)
                nc.vector.tensor_tensor(var[:], var[:], mu2[:], sub_op)
                sd = lnp.tile([128, 1], f32, tag="sd", name=f"sd{qt}")
                nc.scalar.activation(sd[:], var[:], Sqrt, bias=eps_col[:])
                rstd = lnp.tile([128, 1], f32, tag="rstd", name=f"rstd{qt}")
                nc.vector.reciprocal(rstd[:], sd[:])
                bco = lnp.tile([128, 1], f32, tag="bco", name=f"bco{qt}")
                nc.vector.tensor_tensor(bco[:], mu[:], rstd[:], mult_op)
                nc.vector.tensor_scalar_mul(bco[:], bco[:], -1.0)
                ot = lnp.tile([128, D], f32, tag="ot", name=f"ot{qt}")
                nc.vector.tensor_scalar(ot[:], y[:], rstd[:], bco[:],
                                        mult_op, add_op)
                if flags["ln"]:
                    t2 = lnp.tile([128, D], f32, tag="t2", name=f"t2{qt}")
                    nc.vector.tensor_tensor(t2[:], ot[:], gamma_bc[:], mult_op)
                    nc.vector.tensor_tensor(ot[:], t2[:], beta_bc[:], add_op)
                nc.sync.dma_start(out_d[qt * 128:(qt + 1) * 128, :], ot[:])

    nc.compile()
    return nc


def make_in_maps(inputs, cfg=None, flags=None):
    """Build per-core input maps from the full (unsharded) problem inputs."""
    cfg = dict(cfg or FULL_CFG)
    flags = dict(flags or DEFAULT_FLAGS)
    S, D, H, K = cfg["S"], cfg["D"], cfg["H"], cfg["K"]
    HK = H * K
    SH = S // 2
    ND2 = D // 256
    NHK = HK // 128
    NOT = NHK // 2

    def np32(a):
        return np.asarray(a, dtype=np.float32)

    def dpair(w):  # [D, X] -> [ND2, 128, 2, X]
        return np.ascontiguousarray(
            w.reshape(ND2, 2, 128, -1).transpose(0, 2, 1, 3))

    shared = {}
    for nm, key in (("wq_g", "gWq"), ("wk_g", "gWk"), ("wv_g", "gWv"),
                    ("wq_l", "lWq"), ("wk_l", "lWk"), ("wv_l", "lWv")):
        shared[nm] = dpair(np32(inputs[key]).reshape(D, HK) * WSCALE).astype(F8)
    for st, key in (("g", "gWo"), ("l", "lWo")):
        w = np32(inputs[key]).reshape(HK, D) * WSCALE
        shared[f"wo_{st}"] = np.ascontiguousarray(
            w.reshape(NOT, 2, 128, D).transpose(0, 2, 1, 3)).astype(F8)
    for st, q, k in (("g", "gbq", "gbk"), ("l", "lbq", "lbk")):
        if flags[f"bqk_{st}"]:
            shared[f"bq_{st}"] = np.ascontiguousarray(
                np32(inputs[q]).reshape(NHK, 128))
            shared[f"bk_{st}"] = np.ascontiguousarray(
                np32(inputs[k]).reshape(NHK, 128))
    for st, key in (("g", "gbv"), ("l", "lbv")):
        if flags[f"bv_{st}"]:
            shared[f"bv_{st}"] = (np32(inputs[key]).reshape(1, HK)
                                  * WSCALE).astype(F8)
    if flags["ln"]:
        shared["gamma"] = np32(inputs["gamma"]).reshape(1, D)
        shared["beta"] = np32(inputs["beta"]).reshape(1, D)

    x = np32(inputs["x"])
    bo = np32(inputs["gbo"]) + np32(inputs["lbo"])
    in_maps = []
    for c in range(N_CORES):
        b, half = divmod(c, 2)
        xb = x[b]
        # own half first (queries/local), other half second; global attention
        # is invariant to key/value column order
        xperm = np.concatenate([xb[half * SH:(half + 1) * SH],
                                xb[(1 - half) * SH:(2 - half) * SH]], axis=0)
        m = dict(shared)
        m["xT2"] = dpair(np.ascontiguousarray(xperm.T)).astype(F8)
        m["xq"] = np.ascontiguousarray(xperm[0:SH]) + bo
        in_maps.append(m)
    return in_maps


def assemble_out(results, cfg=None):
    cfg = dict(cfg or FULL_CFG)
    S, D = cfg["S"], cfg["D"]
    SH = S // 2
    B = N_CORES // 2
    out = np.empty((B, S, D), np.float32)
    for c in range(N_CORES):
        b, half = divmod(c, 2)
        out[b, half * SH:(half + 1) * SH] = results[c]["out"]
    return out


_NC_CACHE = {}


def kernel(**inputs):
    from concourse.bass_utils import run_bass_kernel_spmd
    flags = flags_for(inputs)
    key = tuple(sorted(flags.items()))
    if key not in _NC_CACHE:
        _NC_CACHE[key] = build_nc(flags=flags)
    nc = _NC_CACHE[key]
    in_maps = make_in_maps(inputs, flags=flags)
    res = run_bass_kernel_spmd(nc, in_maps, list(range(N_CORES)))
    return assemble_out(res.results)
